# revision 2
# baseline (speedup 1.0000x reference)
"""Depthwise Conv3D (3x3x3, VALID, stride 1) on 8 Trainium2 NeuronCores.

Strategy: per-channel Toeplitz matmul over the H axis on TensorE.
  out[b,do,ho,wo,f] = sum_{kd,kh,kw} x[b,do+kd,ho+kh,wo+kw,f] * w[kd,kh,kw,f]
For fixed (f,kd,kw) the sum over kh is a banded [H_in=112, HO=110] Toeplitz
matrix applied along H: one TensorE matmul (contraction over h_in on the
partition dim) handles all 3 kh taps; the 9 (kd,kw) combinations accumulate
in PSUM. Toeplitz matrices are built on the host from the tiny weight tensor.

V2 (bf16):
- x and Toeplitz ship/compute in bf16 (fp32 PSUM accumulation): halves the
  x DMA vs fp32r and drops the even-length/min-256 fp32r ISA restrictions.
- Bias is folded into the matmul: slab row 112 is all-ones and Toeplitz row
  112 holds b[f] on tap 0 only, so PSUM already contains conv+bias and the
  evacuation is a pure copy.
- 16 of 64 channels keep their Toeplitz resident in SBUF (loaded once,
  outside the steady-state loop); the rest stream per 4-channel group.
- W is split into 3 chunks (outputs 38/36/36) so the f32 staging buffer is
  small enough to double-buffer: output drains overlap the next chunk's
  matmuls instead of stalling the PE at chunk boundaries.
- PSUM tiles span 4 banks (one channel per bank); a single evacuation per
  4 channels uses a permuted access pattern whose inner run is 4 floats
  (16B = one SBUF cacheline), ~4x fewer line crossings than per-channel
  strided writes.

Sharding: data-parallel over (batch, D-half) -> 8 shards; weights replicated.
"""

import sys

sys.path.insert(0, "/opt/trn_rl_repo")

from contextlib import ExitStack

import numpy as np

B, D, H, W, F = 4, 16, 112, 112, 64
DO, HO, WO = 14, 110, 110
N_CORES = 8
DO_C = 7  # output d-planes per core
DIN_C = 9  # input d-planes per core
HP = 113  # slab partitions: 112 h rows + ones row (bias tap)

NCH = 3  # W chunks
WINS = [40, 38, 38]  # input w columns per chunk
WEVS = [38, 36, 36]  # output wo columns per chunk
WOFF = [0, 38, 74]  # input w start of each chunk (== output wo start)
PADC = 376  # padded flat (d, w) extent per (h, f); max needed 362
FQ = 4  # channels per PSUM group / streamed-toeplitz DMA
RES_F = 16  # channels with SBUF-resident toeplitz
_NP = 110  # psum partitions (ho)

_cached = None


def _build(loop_n: int = 1):
    from concourse import bacc, mybir, tile

    nc = bacc.Bacc("TRN2", target_bir_lowering=False, debug=False, num_devices=N_CORES)
    f32 = mybir.dt.float32
    bf16 = mybir.dt.bfloat16

    x_ap = nc.dram_tensor("xp", [NCH, HP, F, PADC], bf16, kind="ExternalInput").ap()
    t_ap = nc.dram_tensor("toep", [HP, F, 9, HO], bf16, kind="ExternalInput").ap()
    o_ap = nc.dram_tensor("out", [DO_C, HO, WO, F], f32, kind="ExternalOutput").ap()

    with tile.TileContext(nc) as tc, ExitStack() as ctx:
        res_pool = ctx.enter_context(tc.tile_pool(name="res", bufs=1))
        slab_pool = ctx.enter_context(tc.tile_pool(name="slab", bufs=3))
        toep_pool = ctx.enter_context(tc.tile_pool(name="toep", bufs=2))
        stage_pool = ctx.enter_context(tc.tile_pool(name="stage", bufs=2))
        psum_pool = ctx.enter_context(tc.tile_pool(name="psum", bufs=2, space="PSUM"))

        # resident toeplitz for channels [0, RES_F): loaded once, loop-invariant
        toep_res = res_pool.tile([HP, RES_F, 9, HO], bf16, name="toep_res")
        nc.scalar.dma_start(out=toep_res[:], in_=t_ap[:, 0:RES_F])

        loop_ctx = tc.For_i(0, loop_n) if loop_n > 1 else None
        if loop_ctx is not None:
            ctx.enter_context(loop_ctx)

        for c in range(NCH):
            win, wev, w0 = WINS[c], WEVS[c], WOFF[c]
            nmm = DO_C * win
            stage = stage_pool.tile([_NP, DO_C, wev, F], f32, name="stage", tag="stage")
            for g in range(F // 8):  # slab DMA granularity: 8 channels
                slab_g = slab_pool.tile([HP, 8, PADC], bf16, name="slab_g", tag="sq")
                nc.scalar.dma_start(out=slab_g[:], in_=x_ap[c, :, g * 8 : g * 8 + 8])
                for half in range(2):  # psum-group granularity: 4 channels
                    f0 = g * 8 + half * FQ
                    if f0 + FQ <= RES_F:
                        toep_q, tf0 = toep_res, f0
                    else:
                        toep_q = toep_pool.tile([HP, FQ, 9, HO], bf16, name="toep_q", tag="tq")
                        nc.scalar.dma_start(out=toep_q[:], in_=t_ap[:, f0 : f0 + FQ])
                        tf0 = 0
                    psum4 = psum_pool.tile([_NP, FQ, 512], f32, name="psum4", tag="ps")
                    for fi in range(FQ):
                        for kd in range(3):
                            for kw in range(3):
                                tap = kd * 3 + kw
                                off = kd * win + kw
                                nc.tensor.matmul(
                                    psum4[:, fi, 0:nmm],
                                    lhsT=toep_q[:, tf0 + fi, tap, :],
                                    rhs=slab_g[:, half * FQ + fi, off : off + nmm],
                                    start=(tap == 0),
                                    stop=(tap == 8),
                                )
                    # evacuate 4 channels at once: permute psum [p, f, (d w)]
                    # to [p, d, w, f] so the stage write has 16B inner runs
                    ev_in = (
                        psum4[:, :, 0 : DO_C * win]
                        .rearrange("p f (d w) -> p d w f", d=DO_C)[:, :, 0:wev, :]
                    )
                    ev_out = stage[:, :, :, f0 : f0 + FQ]
                    if half == 0:
                        nc.vector.tensor_copy(ev_out, ev_in)
                    else:
                        nc.scalar.activation(
                            ev_out, ev_in, mybir.ActivationFunctionType.Copy
                        )
            for do in range(DO_C):
                nc.sync.dma_start(
                    out=o_ap[do, :, w0 : w0 + wev, :], in_=stage[:, do]
                )

    nc.compile()
    return nc


def _toeplitz(w: np.ndarray, b: np.ndarray) -> np.ndarray:
    import ml_dtypes

    t = np.zeros((HP, F, 9, HO), np.float32)
    ho = np.arange(HO)
    for kd in range(3):
        for kh in range(3):
            for kw in range(3):
                t[ho + kh, :, kd * 3 + kw, ho] = w[kd, kh, kw, 0, :][None, :]
    t[112, :, 0, :] = b[:, None]  # bias via ones-row on tap 0
    return t.astype(ml_dtypes.bfloat16)


def _pack_x(xs: np.ndarray) -> np.ndarray:
    """[DIN_C, H, W, F] f32 -> [NCH, HP, F, PADC] bf16 slab (chunk, h, f, (d, w))."""
    import ml_dtypes

    xp = np.zeros((NCH, HP, F, PADC), ml_dtypes.bfloat16)
    xp[:, 112, :, :] = 1.0  # ones row: streams the bias tap
    for c in range(NCH):
        win, w0 = WINS[c], WOFF[c]
        chunk = xs[:, :, w0 : w0 + win, :]  # [d, h, w, f]
        xp[c, :112, :, : DIN_C * win] = (
            chunk.transpose(1, 3, 0, 2).reshape(H, F, DIN_C * win).astype(ml_dtypes.bfloat16)
        )
    return xp


def _in_map(core: int, x: np.ndarray, toep: np.ndarray) -> dict:
    bb, dh = divmod(core, 2)
    return {
        "xp": _pack_x(x[bb, dh * DO_C : dh * DO_C + DIN_C]),
        "toep": toep,
    }


def kernel(x: np.ndarray, w: np.ndarray, b: np.ndarray) -> np.ndarray:
    global _cached
    if _cached is None:
        _cached = _build()
    nc = _cached

    from concourse.bass_utils import run_bass_kernel_spmd

    x = np.asarray(x, np.float32)
    toep = _toeplitz(np.asarray(w, np.float32), np.asarray(b, np.float32))
    in_maps = [_in_map(core, x, toep) for core in range(N_CORES)]

    res = run_bass_kernel_spmd(nc, in_maps, list(range(N_CORES)))

    out = np.empty((B, DO, HO, WO, F), np.float32)
    for core in range(N_CORES):
        bb, dh = divmod(core, 2)
        out[bb, dh * DO_C : (dh + 1) * DO_C] = res.results[core]["out"]
    return out


# revision 3
# speedup vs baseline: 8.2854x; 8.2854x over previous
"""Depthwise Conv3D (3x3x3, VALID, stride 1) on 8 Trainium2 NeuronCores.

Strategy: per-channel Toeplitz matmul over the H axis on TensorE.
  out[b,do,ho,wo,f] = sum_{kd,kh,kw} x[b,do+kd,ho+kh,wo+kw,f] * w[kd,kh,kw,f]
For fixed (f,kd,kw) the sum over kh is a banded [H_in=112, HO=110] Toeplitz
matrix applied along H: one TensorE matmul (contraction over h_in on the
partition dim) handles all 3 kh taps; the 9 (kd,kw) combinations accumulate
in PSUM. Toeplitz matrices are built on the host from the tiny weight tensor.

V2 (bf16):
- x and Toeplitz ship/compute in bf16 (fp32 PSUM accumulation): halves the
  x DMA vs fp32r and drops the even-length/min-256 fp32r ISA restrictions.
- Bias is applied during the grouped PSUM evacuation via a broadcast
  tensor_tensor add (stride-0 access pattern on the bias operand).
- 16 of 64 channels keep their Toeplitz resident in SBUF (loaded once,
  outside the steady-state loop); the rest stream per 4-channel group.
- W is split into 3 chunks (outputs 38/36/36) so the f32 staging buffer is
  small enough to double-buffer: output drains overlap the next chunk's
  matmuls instead of stalling the PE at chunk boundaries.
- PSUM tiles span 4 banks (one channel per bank); a single evacuation per
  4 channels uses a permuted access pattern whose inner run is 4 floats
  (16B = one SBUF cacheline), ~4x fewer line crossings than per-channel
  strided writes.

Sharding: data-parallel over (batch, D-half) -> 8 shards; weights replicated.
"""

import sys

sys.path.insert(0, "/opt/trn_rl_repo")

from contextlib import ExitStack

import numpy as np

B, D, H, W, F = 4, 16, 112, 112, 64
DO, HO, WO = 14, 110, 110
N_CORES = 8
DO_C = 7  # output d-planes per core
DIN_C = 9  # input d-planes per core
HP = 112  # slab/toeplitz partitions (112 h rows; 16-aligned for DMA spread)

NCH = 3  # W chunks
WINS = [40, 38, 38]  # input w columns per chunk
WEVS = [38, 36, 36]  # output wo columns per chunk
WOFF = [0, 38, 74]  # input w start of each chunk (== output wo start)
PADC = 376  # padded flat (d, w) extent per (h, f); max needed 362
FQ = 4  # channels per PSUM group / streamed-toeplitz DMA
RES_F = 16  # channels with SBUF-resident toeplitz
_NP = 110  # psum partitions (ho)

_cached = None


def _build(loop_n: int = 1):
    from concourse import bacc, mybir, tile

    nc = bacc.Bacc("TRN2", target_bir_lowering=False, debug=False, num_devices=N_CORES)
    f32 = mybir.dt.float32
    bf16 = mybir.dt.bfloat16

    x_ap = nc.dram_tensor("xp", [NCH, HP, F, PADC], bf16, kind="ExternalInput").ap()
    t_ap = nc.dram_tensor("toep", [HP, F, 9, HO], bf16, kind="ExternalInput").ap()
    b_ap = nc.dram_tensor("biasbc", [128, F], f32, kind="ExternalInput").ap()
    o_ap = nc.dram_tensor("out", [DO_C, HO, WO, F], f32, kind="ExternalOutput").ap()

    with tile.TileContext(nc) as tc, ExitStack() as ctx:
        res_pool = ctx.enter_context(tc.tile_pool(name="res", bufs=1))
        slab_pool = ctx.enter_context(tc.tile_pool(name="slab", bufs=3))
        toep_pool = ctx.enter_context(tc.tile_pool(name="toep", bufs=2))
        stage_pool = ctx.enter_context(tc.tile_pool(name="stage", bufs=2))
        psum_pool = ctx.enter_context(tc.tile_pool(name="psum", bufs=2, space="PSUM"))

        # resident toeplitz for channels [0, RES_F) + bias: loaded once
        toep_res = res_pool.tile([HP, RES_F, 9, HO], bf16, name="toep_res")
        nc.scalar.dma_start(out=toep_res[:], in_=t_ap[:, 0:RES_F])
        bias_t = res_pool.tile([128, F], f32, name="bias_t")
        nc.scalar.dma_start(out=bias_t[:], in_=b_ap[:])

        loop_ctx = tc.For_i(0, loop_n) if loop_n > 1 else None
        if loop_ctx is not None:
            ctx.enter_context(loop_ctx)

        for c in range(NCH):
            win, wev, w0 = WINS[c], WEVS[c], WOFF[c]
            nmm = DO_C * win
            stage = stage_pool.tile([_NP, DO_C, wev, F], f32, name="stage", tag="stage")
            for g in range(F // 8):  # slab DMA granularity: 8 channels
                slab_g = slab_pool.tile([HP, 8, PADC], bf16, name="slab_g", tag="sq")
                nc.scalar.dma_start(out=slab_g[:], in_=x_ap[c, :, g * 8 : g * 8 + 8])
                for half in range(2):  # psum-group granularity: 4 channels
                    f0 = g * 8 + half * FQ
                    if f0 + FQ <= RES_F:
                        toep_q, tf0 = toep_res, f0
                    else:
                        toep_q = toep_pool.tile([HP, FQ, 9, HO], bf16, name="toep_q", tag="tq")
                        nc.scalar.dma_start(out=toep_q[:], in_=t_ap[:, f0 : f0 + FQ])
                        tf0 = 0
                    psum4 = psum_pool.tile([_NP, FQ, 512], f32, name="psum4", tag="ps")
                    for fi in range(FQ):
                        for kd in range(3):
                            for kw in range(3):
                                tap = kd * 3 + kw
                                off = kd * win + kw
                                nc.tensor.matmul(
                                    psum4[:, fi, 0:nmm],
                                    lhsT=toep_q[:, tf0 + fi, tap, :],
                                    rhs=slab_g[:, half * FQ + fi, off : off + nmm],
                                    start=(tap == 0),
                                    stop=(tap == 8),
                                )
                    # evacuate 4 channels at once: permute psum [p, f, (d w)]
                    # to [p, d, w, f] so the stage write has 16B inner runs
                    ev_in = (
                        psum4[:, :, 0 : DO_C * win]
                        .rearrange("p f (d w) -> p d w f", d=DO_C)[:, :, 0:wev, :]
                    )
                    ev_out = stage[:, :, :, f0 : f0 + FQ]
                    ev_b = (
                        bias_t[0:_NP, f0 : f0 + FQ]
                        .unsqueeze(1)
                        .unsqueeze(2)
                        .broadcast_to([_NP, DO_C, wev, FQ])
                    )
                    nc.vector.tensor_tensor(
                        ev_out, ev_in, ev_b, mybir.AluOpType.add
                    )
            for do in range(DO_C):
                nc.sync.dma_start(
                    out=o_ap[do, :, w0 : w0 + wev, :], in_=stage[:, do]
                )

    nc.compile()
    return nc


def _toeplitz(w: np.ndarray) -> np.ndarray:
    import ml_dtypes

    t = np.zeros((HP, F, 9, HO), np.float32)
    ho = np.arange(HO)
    for kd in range(3):
        for kh in range(3):
            for kw in range(3):
                t[ho + kh, :, kd * 3 + kw, ho] = w[kd, kh, kw, 0, :][None, :]
    return t.astype(ml_dtypes.bfloat16)


def _pack_x(xs: np.ndarray) -> np.ndarray:
    """[DIN_C, H, W, F] f32 -> [NCH, HP, F, PADC] bf16 slab (chunk, h, f, (d, w))."""
    import ml_dtypes

    xp = np.zeros((NCH, HP, F, PADC), ml_dtypes.bfloat16)
    for c in range(NCH):
        win, w0 = WINS[c], WOFF[c]
        chunk = xs[:, :, w0 : w0 + win, :]  # [d, h, w, f]
        xp[c, :, :, : DIN_C * win] = (
            chunk.transpose(1, 3, 0, 2).reshape(H, F, DIN_C * win).astype(ml_dtypes.bfloat16)
        )
    return xp


def _in_map(core: int, x: np.ndarray, toep: np.ndarray, bias_bc: np.ndarray) -> dict:
    bb, dh = divmod(core, 2)
    return {
        "xp": _pack_x(x[bb, dh * DO_C : dh * DO_C + DIN_C]),
        "toep": toep,
        "biasbc": bias_bc,
    }


def kernel(x: np.ndarray, w: np.ndarray, b: np.ndarray) -> np.ndarray:
    global _cached
    if _cached is None:
        _cached = _build()
    nc = _cached

    from concourse.bass_utils import run_bass_kernel_spmd

    x = np.asarray(x, np.float32)
    toep = _toeplitz(np.asarray(w, np.float32))
    bias_bc = np.tile(np.asarray(b, np.float32)[None, :], (128, 1))
    in_maps = [_in_map(core, x, toep, bias_bc) for core in range(N_CORES)]

    res = run_bass_kernel_spmd(nc, in_maps, list(range(N_CORES)))

    out = np.empty((B, DO, HO, WO, F), np.float32)
    for core in range(N_CORES):
        bb, dh = divmod(core, 2)
        out[bb, dh * DO_C : (dh + 1) * DO_C] = res.results[core]["out"]
    return out


# revision 4
# speedup vs baseline: 9.7136x; 1.1724x over previous
"""Depthwise Conv3D (3x3x3, VALID, stride 1) on 8 Trainium2 NeuronCores.

Strategy: per-channel Toeplitz matmul over the H axis on TensorE.
  out[b,do,ho,wo,f] = sum_{kd,kh,kw} x[b,do+kd,ho+kh,wo+kw,f] * w[kd,kh,kw,f]
For fixed (f,kd,kw) the sum over kh is a banded [H_in=112, HO=110] Toeplitz
matrix applied along H: one TensorE matmul (contraction over h_in on the
partition dim) handles all 3 kh taps; the 9 (kd,kw) combinations accumulate
in PSUM. Toeplitz matrices are built on the host from the tiny weight tensor.

V2 (bf16):
- x and Toeplitz ship/compute in bf16 (fp32 PSUM accumulation): halves the
  x DMA vs fp32r and drops the even-length/min-256 fp32r ISA restrictions.
- Bias is applied during the grouped PSUM evacuation via a broadcast
  tensor_tensor add (stride-0 access pattern on the bias operand).
- Toeplitz streams per 8-channel group on the sync ring (3 buffers deep,
  decoupled from the slab FIFO on the scalar ring).
- W is split into 3 chunks (outputs 38/36/36) so the f32 staging buffer is
  small enough to double-buffer: output drains overlap the next chunk's
  matmuls instead of stalling the PE at chunk boundaries.
- PSUM tiles span 4 banks (one channel per bank); a single evacuation per
  4 channels uses a permuted access pattern whose inner run is 4 floats
  (16B = one SBUF cacheline), ~4x fewer line crossings than per-channel
  strided writes.

Sharding: data-parallel over (batch, D-half) -> 8 shards; weights replicated.
"""

import sys

sys.path.insert(0, "/opt/trn_rl_repo")

from contextlib import ExitStack

import numpy as np

B, D, H, W, F = 4, 16, 112, 112, 64
DO, HO, WO = 14, 110, 110
N_CORES = 8
DO_C = 7  # output d-planes per core
DIN_C = 9  # input d-planes per core
HP = 112  # slab/toeplitz partitions (112 h rows; 16-aligned for DMA spread)

NCH = 3  # W chunks
WINS = [40, 38, 38]  # input w columns per chunk
WEVS = [38, 36, 36]  # output wo columns per chunk
WOFF = [0, 38, 74]  # input w start of each chunk (== output wo start)
PADC = 376  # padded flat (d, w) extent per (h, f); max needed 362
FQ = 4  # channels per PSUM group
_NP = 110  # psum partitions (ho)
UNROLL = 3  # steady-state iterations per hardware-loop body

_cached = None


def _build(loop_n: int = 1):
    from concourse import bacc, mybir, tile

    nc = bacc.Bacc("TRN2", target_bir_lowering=False, debug=False, num_devices=N_CORES)
    f32 = mybir.dt.float32
    bf16 = mybir.dt.bfloat16

    x_ap = nc.dram_tensor("xp", [NCH, HP, F, PADC], bf16, kind="ExternalInput").ap()
    t_ap = nc.dram_tensor("toep", [HP, F, 9, HO], bf16, kind="ExternalInput").ap()
    b_ap = nc.dram_tensor("biasbc", [128, F], f32, kind="ExternalInput").ap()
    o_ap = nc.dram_tensor("out", [DO_C, HO, WO, F], f32, kind="ExternalOutput").ap()

    with tile.TileContext(nc) as tc, ExitStack() as ctx:
        res_pool = ctx.enter_context(tc.tile_pool(name="res", bufs=1))
        slab_pool = ctx.enter_context(tc.tile_pool(name="slab", bufs=4))
        toep_pool = ctx.enter_context(tc.tile_pool(name="toep", bufs=3))
        stage_pool = ctx.enter_context(tc.tile_pool(name="stage", bufs=2))
        psum_pool = ctx.enter_context(tc.tile_pool(name="psum", bufs=2, space="PSUM"))

        bias_t = res_pool.tile([128, F], f32, name="bias_t")
        nc.scalar.dma_start(out=bias_t[:], in_=b_ap[:])

        def body():
            for c in range(NCH):
                win, wev, w0 = WINS[c], WEVS[c], WOFF[c]
                nmm = DO_C * win
                stage = stage_pool.tile(
                    [_NP, DO_C, wev, F], f32, name="stage", tag="stage"
                )
                for g in range(F // 8):  # slab/toeplitz DMA granularity: 8 ch
                    slab_g = slab_pool.tile([HP, 8, PADC], bf16, name="slab_g", tag="sq")
                    nc.scalar.dma_start(out=slab_g[:], in_=x_ap[c, :, g * 8 : g * 8 + 8])
                    toep_g = toep_pool.tile([HP, 8, 9, HO], bf16, name="toep_g", tag="tq")
                    nc.sync.dma_start(out=toep_g[:], in_=t_ap[:, g * 8 : g * 8 + 8])
                    for half in range(2):  # psum-group granularity: 4 channels
                        f0 = g * 8 + half * FQ
                        psum4 = psum_pool.tile([_NP, FQ, 512], f32, name="psum4", tag="ps")
                        for fi in range(FQ):
                            for kd in range(3):
                                for kw in range(3):
                                    tap = kd * 3 + kw
                                    off = kd * win + kw
                                    nc.tensor.matmul(
                                        psum4[:, fi, 0:nmm],
                                        lhsT=toep_g[:, half * FQ + fi, tap, :],
                                        rhs=slab_g[:, half * FQ + fi, off : off + nmm],
                                        start=(tap == 0),
                                        stop=(tap == 8),
                                    )
                        # evacuate 4 channels at once: permute psum [p, f, (d w)]
                        # to [p, d, w, f] so the stage write has 16B inner runs
                        ev_in = (
                            psum4[:, :, 0 : DO_C * win]
                            .rearrange("p f (d w) -> p d w f", d=DO_C)[:, :, 0:wev, :]
                        )
                        ev_out = stage[:, :, :, f0 : f0 + FQ]
                        ev_b = (
                            bias_t[0:_NP, f0 : f0 + FQ]
                            .unsqueeze(1)
                            .unsqueeze(2)
                            .broadcast_to([_NP, DO_C, wev, FQ])
                        )
                        nc.vector.tensor_tensor(
                            ev_out, ev_in, ev_b, mybir.AluOpType.add
                        )
                for do in range(DO_C):
                    nc.sync.dma_start(
                        out=o_ap[do, :, w0 : w0 + wev, :], in_=stage[:, do]
                    )

        # unroll UNROLL bodies per hardware-loop trip to amortize the
        # loop-edge barrier (engine drain + semaphore reset)
        n_loop = loop_n // UNROLL
        pre = loop_n - n_loop * UNROLL
        if n_loop == 1:
            pre, n_loop = loop_n, 0
        for _ in range(pre):
            body()
        if n_loop >= 2:
            with tc.For_i(0, n_loop):
                for _ in range(UNROLL):
                    body()

    nc.compile()
    return nc


def _toeplitz(w: np.ndarray) -> np.ndarray:
    import ml_dtypes

    t = np.zeros((HP, F, 9, HO), np.float32)
    ho = np.arange(HO)
    for kd in range(3):
        for kh in range(3):
            for kw in range(3):
                t[ho + kh, :, kd * 3 + kw, ho] = w[kd, kh, kw, 0, :][None, :]
    return t.astype(ml_dtypes.bfloat16)


def _pack_x(xs: np.ndarray) -> np.ndarray:
    """[DIN_C, H, W, F] f32 -> [NCH, HP, F, PADC] bf16 slab (chunk, h, f, (d, w))."""
    import ml_dtypes

    xp = np.zeros((NCH, HP, F, PADC), ml_dtypes.bfloat16)
    for c in range(NCH):
        win, w0 = WINS[c], WOFF[c]
        chunk = xs[:, :, w0 : w0 + win, :]  # [d, h, w, f]
        xp[c, :, :, : DIN_C * win] = (
            chunk.transpose(1, 3, 0, 2).reshape(H, F, DIN_C * win).astype(ml_dtypes.bfloat16)
        )
    return xp


def _in_map(core: int, x: np.ndarray, toep: np.ndarray, bias_bc: np.ndarray) -> dict:
    bb, dh = divmod(core, 2)
    return {
        "xp": _pack_x(x[bb, dh * DO_C : dh * DO_C + DIN_C]),
        "toep": toep,
        "biasbc": bias_bc,
    }


def kernel(x: np.ndarray, w: np.ndarray, b: np.ndarray) -> np.ndarray:
    global _cached
    if _cached is None:
        _cached = _build()
    nc = _cached

    from concourse.bass_utils import run_bass_kernel_spmd

    x = np.asarray(x, np.float32)
    toep = _toeplitz(np.asarray(w, np.float32))
    bias_bc = np.tile(np.asarray(b, np.float32)[None, :], (128, 1))
    in_maps = [_in_map(core, x, toep, bias_bc) for core in range(N_CORES)]

    res = run_bass_kernel_spmd(nc, in_maps, list(range(N_CORES)))

    out = np.empty((B, DO, HO, WO, F), np.float32)
    for core in range(N_CORES):
        bb, dh = divmod(core, 2)
        out[bb, dh * DO_C : (dh + 1) * DO_C] = res.results[core]["out"]
    return out


# revision 5
# speedup vs baseline: 14.7967x; 1.5233x over previous
"""Depthwise Conv3D (3x3x3, VALID, stride 1) on 8 Trainium2 NeuronCores.

Strategy: per-channel Toeplitz matmul over the H axis on TensorE.
  out[b,do,ho,wo,f] = sum_{kd,kh,kw} x[b,do+kd,ho+kh,wo+kw,f] * w[kd,kh,kw,f]
For fixed (f,kd,kw) the sum over kh is a banded [H_in=112, HO=110] Toeplitz
matrix applied along H: one TensorE matmul (contraction over h_in on the
partition dim) handles all 3 kh taps; the 9 (kd,kw) combinations accumulate
in PSUM. Toeplitz matrices are built on the host from the tiny weight tensor.

V2 (bf16):
- x and Toeplitz ship/compute in bf16 (fp32 PSUM accumulation): halves the
  x DMA vs fp32r and drops the even-length/min-256 fp32r ISA restrictions.
- Bias is applied during the grouped PSUM evacuation via a broadcast
  tensor_tensor add (stride-0 access pattern on the bias operand).
- The full Toeplitz (64ch, 126.7KB/partition bf16) is SBUF-resident, loaded
  once outside the steady-state loop: zero weight DMA per iteration.
- The staging buffer is bf16; output drains are gpsimd SWDGE cast-DMAs
  (bf16 SBUF -> f32 DRAM), halving stage SBUF so it still double-buffers.
- W is split into 3 chunks (outputs 38/36/36) so the f32 staging buffer is
  small enough to double-buffer: output drains overlap the next chunk's
  matmuls instead of stalling the PE at chunk boundaries.
- PSUM tiles span 4 banks (one channel per bank); a single evacuation per
  4 channels uses a permuted access pattern whose inner run is 4 floats
  (16B = one SBUF cacheline), ~4x fewer line crossings than per-channel
  strided writes.

Sharding: data-parallel over (batch, D-half) -> 8 shards; weights replicated.
"""

import sys

sys.path.insert(0, "/opt/trn_rl_repo")

from contextlib import ExitStack

import numpy as np

B, D, H, W, F = 4, 16, 112, 112, 64
DO, HO, WO = 14, 110, 110
N_CORES = 8
DO_C = 7  # output d-planes per core
DIN_C = 9  # input d-planes per core
HP = 112  # slab/toeplitz partitions (112 h rows; 16-aligned for DMA spread)

NCH = 3  # W chunks
WINS = [40, 38, 38]  # input w columns per chunk
WEVS = [38, 36, 36]  # output wo columns per chunk
WOFF = [0, 38, 74]  # input w start of each chunk (== output wo start)
PADC = 376  # padded flat (d, w) extent per (h, f); max needed 362
FQ = 4  # channels per PSUM group
_NP = 110  # psum partitions (ho)
UNROLL = 3  # steady-state iterations per hardware-loop body

_cached = None


def _build(loop_n: int = 1):
    from concourse import bacc, mybir, tile

    nc = bacc.Bacc("TRN2", target_bir_lowering=False, debug=False, num_devices=N_CORES)
    f32 = mybir.dt.float32
    bf16 = mybir.dt.bfloat16

    x_ap = nc.dram_tensor("xp", [NCH, HP, F, PADC], bf16, kind="ExternalInput").ap()
    t_ap = nc.dram_tensor("toep", [HP, F, 9, HO], bf16, kind="ExternalInput").ap()
    b_ap = nc.dram_tensor("biasbc", [128, F], f32, kind="ExternalInput").ap()
    o_ap = nc.dram_tensor("out", [DO_C, HO, WO, F], f32, kind="ExternalOutput").ap()

    with tile.TileContext(nc) as tc, ExitStack() as ctx:
        res_pool = ctx.enter_context(tc.tile_pool(name="res", bufs=1))
        slab_pool = ctx.enter_context(tc.tile_pool(name="slab", bufs=2))
        stage_pool = ctx.enter_context(tc.tile_pool(name="stage", bufs=2))
        psum_pool = ctx.enter_context(tc.tile_pool(name="psum", bufs=2, space="PSUM"))

        bias_t = res_pool.tile([128, F], f32, name="bias_t")
        nc.scalar.dma_start(out=bias_t[:], in_=b_ap[:])
        toep_res = res_pool.tile([HP, F, 9, HO], bf16, name="toep_res")
        nc.scalar.dma_start(out=toep_res[:], in_=t_ap[:])

        def body():
            for c in range(NCH):
                win, wev, w0 = WINS[c], WEVS[c], WOFF[c]
                nmm = DO_C * win
                stage = stage_pool.tile(
                    [_NP, DO_C, wev, F], bf16, name="stage", tag="stage"
                )
                for g in range(F // 8):  # slab DMA granularity: 8 channels
                    slab_g = slab_pool.tile([HP, 8, PADC], bf16, name="slab_g", tag="sq")
                    nc.scalar.dma_start(out=slab_g[:], in_=x_ap[c, :, g * 8 : g * 8 + 8])
                    for half in range(2):  # psum-group granularity: 4 channels
                        f0 = g * 8 + half * FQ
                        psum4 = psum_pool.tile([_NP, FQ, 512], f32, name="psum4", tag="ps")
                        for fi in range(FQ):
                            for kd in range(3):
                                for kw in range(3):
                                    tap = kd * 3 + kw
                                    off = kd * win + kw
                                    nc.tensor.matmul(
                                        psum4[:, fi, 0:nmm],
                                        lhsT=toep_res[:, f0 + fi, tap, :],
                                        rhs=slab_g[:, half * FQ + fi, off : off + nmm],
                                        start=(tap == 0),
                                        stop=(tap == 8),
                                    )
                        # evacuate 4 channels at once: permute psum [p, f, (d w)]
                        # to [p, d, w, f] so the stage write has 16B inner runs
                        ev_in = (
                            psum4[:, :, 0 : DO_C * win]
                            .rearrange("p f (d w) -> p d w f", d=DO_C)[:, :, 0:wev, :]
                        )
                        ev_out = stage[:, :, :, f0 : f0 + FQ]
                        ev_b = (
                            bias_t[0:_NP, f0 : f0 + FQ]
                            .unsqueeze(1)
                            .unsqueeze(2)
                            .broadcast_to([_NP, DO_C, wev, FQ])
                        )
                        nc.vector.tensor_tensor(
                            ev_out, ev_in, ev_b, mybir.AluOpType.add
                        )
                for do in range(DO_C):
                    # SWDGE cast-DMA: bf16 stage -> f32 DRAM output
                    nc.gpsimd.dma_start(
                        out=o_ap[do, :, w0 : w0 + wev, :], in_=stage[:, do]
                    )

        # unroll UNROLL bodies per hardware-loop trip to amortize the
        # loop-edge barrier (engine drain + semaphore reset)
        n_loop = loop_n // UNROLL
        pre = loop_n - n_loop * UNROLL
        if n_loop == 1:
            pre, n_loop = loop_n, 0
        for _ in range(pre):
            body()
        if n_loop >= 2:
            with tc.For_i(0, n_loop):
                for _ in range(UNROLL):
                    body()

    nc.compile()
    return nc


def _toeplitz(w: np.ndarray) -> np.ndarray:
    import ml_dtypes

    t = np.zeros((HP, F, 9, HO), np.float32)
    ho = np.arange(HO)
    for kd in range(3):
        for kh in range(3):
            for kw in range(3):
                t[ho + kh, :, kd * 3 + kw, ho] = w[kd, kh, kw, 0, :][None, :]
    return t.astype(ml_dtypes.bfloat16)


def _pack_x(xs: np.ndarray) -> np.ndarray:
    """[DIN_C, H, W, F] f32 -> [NCH, HP, F, PADC] bf16 slab (chunk, h, f, (d, w))."""
    import ml_dtypes

    xp = np.zeros((NCH, HP, F, PADC), ml_dtypes.bfloat16)
    for c in range(NCH):
        win, w0 = WINS[c], WOFF[c]
        chunk = xs[:, :, w0 : w0 + win, :]  # [d, h, w, f]
        xp[c, :, :, : DIN_C * win] = (
            chunk.transpose(1, 3, 0, 2).reshape(H, F, DIN_C * win).astype(ml_dtypes.bfloat16)
        )
    return xp


def _in_map(core: int, x: np.ndarray, toep: np.ndarray, bias_bc: np.ndarray) -> dict:
    bb, dh = divmod(core, 2)
    return {
        "xp": _pack_x(x[bb, dh * DO_C : dh * DO_C + DIN_C]),
        "toep": toep,
        "biasbc": bias_bc,
    }


def kernel(x: np.ndarray, w: np.ndarray, b: np.ndarray) -> np.ndarray:
    global _cached
    if _cached is None:
        _cached = _build()
    nc = _cached

    from concourse.bass_utils import run_bass_kernel_spmd

    x = np.asarray(x, np.float32)
    toep = _toeplitz(np.asarray(w, np.float32))
    bias_bc = np.tile(np.asarray(b, np.float32)[None, :], (128, 1))
    in_maps = [_in_map(core, x, toep, bias_bc) for core in range(N_CORES)]

    res = run_bass_kernel_spmd(nc, in_maps, list(range(N_CORES)))

    out = np.empty((B, DO, HO, WO, F), np.float32)
    for core in range(N_CORES):
        bb, dh = divmod(core, 2)
        out[bb, dh * DO_C : (dh + 1) * DO_C] = res.results[core]["out"]
    return out


# revision 6
# speedup vs baseline: 20.9070x; 1.4130x over previous
"""Depthwise Conv3D (3x3x3, VALID, stride 1) on 8 Trainium2 NeuronCores.

Strategy: per-channel Toeplitz matmul over the H axis on TensorE.
  out[b,do,ho,wo,f] = sum_{kd,kh,kw} x[b,do+kd,ho+kh,wo+kw,f] * w[kd,kh,kw,f]
For fixed (f,kd,kw) the sum over kh is a banded [H_in=112, HO=110] Toeplitz
matrix applied along H: one TensorE matmul (contraction over h_in on the
partition dim) handles all 3 kh taps; the 9 (kd,kw) combinations accumulate
in PSUM. Toeplitz matrices are built on the host from the tiny weight tensor.

V2 (bf16):
- x and Toeplitz ship/compute in bf16 (fp32 PSUM accumulation): halves the
  x DMA vs fp32r and drops the even-length/min-256 fp32r ISA restrictions.
- Bias is applied during the grouped PSUM evacuation via a broadcast
  tensor_tensor add (stride-0 access pattern on the bias operand).
- The full Toeplitz (64ch, 126.7KB/partition bf16) is SBUF-resident, loaded
  once outside the steady-state loop: zero weight DMA per iteration.
- The staging buffer is bf16; output drains are gpsimd SWDGE cast-DMAs
  (bf16 SBUF -> f32 DRAM), halving stage SBUF so it still double-buffers.
- W is split into 3 chunks (outputs 38/36/36) so the f32 staging buffer is
  small enough to double-buffer: output drains overlap the next chunk's
  matmuls instead of stalling the PE at chunk boundaries.
- PSUM tiles span 4 banks (one channel per bank); a single evacuation per
  4 channels uses a permuted access pattern whose inner run is 4 floats
  (16B = one SBUF cacheline), ~4x fewer line crossings than per-channel
  strided writes.

Sharding: data-parallel over (batch, D-half) -> 8 shards; weights replicated.
"""

import sys

sys.path.insert(0, "/opt/trn_rl_repo")

from contextlib import ExitStack

import numpy as np

B, D, H, W, F = 4, 16, 112, 112, 64
DO, HO, WO = 14, 110, 110
N_CORES = 8
DO_C = 7  # output d-planes per core
DIN_C = 9  # input d-planes per core
HP = 112  # slab/toeplitz partitions (112 h rows; 16-aligned for DMA spread)

NCH = 3  # W chunks
WINS = [40, 38, 38]  # input w columns per chunk
WEVS = [38, 36, 36]  # output wo columns per chunk
WOFF = [0, 38, 74]  # input w start of each chunk (== output wo start)
PADC = 368  # padded flat (d, w) extent per (h, f); max needed 362
FQ = 4  # channels per PSUM group
_NP = 110  # psum partitions (ho)
UNROLL = 7  # steady-state iterations per hardware-loop body

_cached = None


def _build(loop_n: int = 1):
    from concourse import bacc, mybir, tile

    nc = bacc.Bacc("TRN2", target_bir_lowering=False, debug=False, num_devices=N_CORES)
    f32 = mybir.dt.float32
    bf16 = mybir.dt.bfloat16

    x_ap = nc.dram_tensor("xp", [NCH, HP, F, PADC], bf16, kind="ExternalInput").ap()
    t_ap = nc.dram_tensor("toep", [HP, F, 9, HO], bf16, kind="ExternalInput").ap()
    b_ap = nc.dram_tensor("biasbc", [128, F], f32, kind="ExternalInput").ap()
    o_ap = nc.dram_tensor("out", [DO_C, HO, WO, F], f32, kind="ExternalOutput").ap()

    with tile.TileContext(nc) as tc, ExitStack() as ctx:
        res_pool = ctx.enter_context(tc.tile_pool(name="res", bufs=1))
        slab_pool = ctx.enter_context(tc.tile_pool(name="slab", bufs=3))
        stage_pool = ctx.enter_context(tc.tile_pool(name="stage", bufs=2))
        psum_pool = ctx.enter_context(tc.tile_pool(name="psum", bufs=2, space="PSUM"))

        bias_t = res_pool.tile([128, F], f32, name="bias_t")
        nc.scalar.dma_start(out=bias_t[:], in_=b_ap[:])
        toep_res = res_pool.tile([HP, F, 9, HO], bf16, name="toep_res")
        nc.scalar.dma_start(out=toep_res[:], in_=t_ap[:])

        def body():
            for c in range(NCH):
                win, wev, w0 = WINS[c], WEVS[c], WOFF[c]
                nmm = DO_C * win
                stage = stage_pool.tile(
                    [_NP, DO_C, wev, F], bf16, name="stage", tag="stage"
                )
                for g in range(F // 8):  # slab DMA granularity: 8 channels
                    slab_g = slab_pool.tile([HP, 8, PADC], bf16, name="slab_g", tag="sq")
                    nc.scalar.dma_start(out=slab_g[:], in_=x_ap[c, :, g * 8 : g * 8 + 8])
                    for half in range(2):  # psum-group granularity: 4 channels
                        f0 = g * 8 + half * FQ
                        psum4 = psum_pool.tile([_NP, FQ, 512], f32, name="psum4", tag="ps")
                        for fi in range(FQ):
                            for kd in range(3):
                                for kw in range(3):
                                    tap = kd * 3 + kw
                                    off = kd * win + kw
                                    nc.tensor.matmul(
                                        psum4[:, fi, 0:nmm],
                                        lhsT=toep_res[:, f0 + fi, tap, :],
                                        rhs=slab_g[:, half * FQ + fi, off : off + nmm],
                                        start=(tap == 0),
                                        stop=(tap == 8),
                                    )
                        # evacuate 4 channels at once: permute psum [p, f, (d w)]
                        # to [p, d, w, f] so the stage write has 16B inner runs
                        ev_in = (
                            psum4[:, :, 0 : DO_C * win]
                            .rearrange("p f (d w) -> p d w f", d=DO_C)[:, :, 0:wev, :]
                        )
                        ev_out = stage[:, :, :, f0 : f0 + FQ]
                        ev_b = (
                            bias_t[0:_NP, f0 : f0 + FQ]
                            .unsqueeze(1)
                            .unsqueeze(2)
                            .broadcast_to([_NP, DO_C, wev, FQ])
                        )
                        nc.vector.tensor_tensor(
                            ev_out, ev_in, ev_b, mybir.AluOpType.add
                        )
                for do in range(DO_C):
                    # SWDGE cast-DMA: bf16 stage -> f32 DRAM output
                    nc.gpsimd.dma_start(
                        out=o_ap[do, :, w0 : w0 + wev, :], in_=stage[:, do]
                    )

        # unroll UNROLL bodies per hardware-loop trip to amortize the
        # loop-edge barrier (engine drain + semaphore reset)
        n_loop = loop_n // UNROLL
        pre = loop_n - n_loop * UNROLL
        if n_loop == 1:
            pre, n_loop = loop_n, 0
        for _ in range(pre):
            body()
        if n_loop >= 2:
            with tc.For_i(0, n_loop):
                for _ in range(UNROLL):
                    body()

    nc.compile()
    return nc


def _toeplitz(w: np.ndarray) -> np.ndarray:
    import ml_dtypes

    t = np.zeros((HP, F, 9, HO), np.float32)
    ho = np.arange(HO)
    for kd in range(3):
        for kh in range(3):
            for kw in range(3):
                t[ho + kh, :, kd * 3 + kw, ho] = w[kd, kh, kw, 0, :][None, :]
    return t.astype(ml_dtypes.bfloat16)


def _pack_x(xs: np.ndarray) -> np.ndarray:
    """[DIN_C, H, W, F] f32 -> [NCH, HP, F, PADC] bf16 slab (chunk, h, f, (d, w))."""
    import ml_dtypes

    xp = np.zeros((NCH, HP, F, PADC), ml_dtypes.bfloat16)
    for c in range(NCH):
        win, w0 = WINS[c], WOFF[c]
        chunk = xs[:, :, w0 : w0 + win, :]  # [d, h, w, f]
        xp[c, :, :, : DIN_C * win] = (
            chunk.transpose(1, 3, 0, 2).reshape(H, F, DIN_C * win).astype(ml_dtypes.bfloat16)
        )
    return xp


def _in_map(core: int, x: np.ndarray, toep: np.ndarray, bias_bc: np.ndarray) -> dict:
    bb, dh = divmod(core, 2)
    return {
        "xp": _pack_x(x[bb, dh * DO_C : dh * DO_C + DIN_C]),
        "toep": toep,
        "biasbc": bias_bc,
    }


def kernel(x: np.ndarray, w: np.ndarray, b: np.ndarray) -> np.ndarray:
    global _cached
    if _cached is None:
        _cached = _build()
    nc = _cached

    from concourse.bass_utils import run_bass_kernel_spmd

    x = np.asarray(x, np.float32)
    toep = _toeplitz(np.asarray(w, np.float32))
    bias_bc = np.tile(np.asarray(b, np.float32)[None, :], (128, 1))
    in_maps = [_in_map(core, x, toep, bias_bc) for core in range(N_CORES)]

    res = run_bass_kernel_spmd(nc, in_maps, list(range(N_CORES)))

    out = np.empty((B, DO, HO, WO, F), np.float32)
    for core in range(N_CORES):
        bb, dh = divmod(core, 2)
        out[bb, dh * DO_C : (dh + 1) * DO_C] = res.results[core]["out"]
    return out
